# revision 24
# baseline (speedup 1.0000x reference)
"""Mistral GQA attention on 8 Trainium2 NeuronCores (bf16 v2).

Sharding: core c -> batch b = c//4, head-group g = c%4.
Each core computes 8 query heads (g*8..g*8+8) and their 2 KV heads
(g*2, g*2+1) for its batch over the full sequence, plus the partial
output projection for its head rows of Wo. Host sums the 4 partial
outputs per batch.

Optimizations vs the fp32r baseline:
- all matmul operands bf16 (fp32 PSUM accumulation); host converts.
  Same 1 cycle/row PE rate as fp32r@>=256-wide, but halves DMA/SBUF
  and removes the <256-wide 4x penalty on diagonal tiles.
- merged multi-et DMA transfers via 4D host layouts (HWDGE charges a
  fixed ~625ns per transfer on one shared device), split across the
  SP / ACT(HWDGE) and Pool(SWDGE) queues to avoid head-of-line blocks.
- every phase is co-scheduled with a complementary phase so the PE
  never idles (idle resets the PE p-state: 2x slower rows for 3us):
  A-k inside A-q half 0; C half 0 inside A-q half 1; A-v stages inside
  the (ACT-exp-bound) attention blocks b0-b2; C half 1 q-block 2
  inside b3. PSUM is split into tags A(4)/B(2)/C(2) so co-scheduled
  phases own disjoint banks and drains overlap accumulation.
- softmax denominator via DVE accumulation of exp tiles + one
  ones-matmul per (head, q-block) instead of a ones-matmul per k-tile;
  attention heads run in pairs to cover the exp chain latency.
- wv/xq-half resident in SBUF (no x re-reads); bf16 output partials
  summed on host.
"""
import numpy as np

B, S, E = 2, 2048, 4096
H, KVH, D = 32, 8, 128
GROUPS = H // KVH
N_CORES, TP = 8, 4
HQ = H // TP          # 8 q heads per core
NKV = KVH // TP       # 2 kv heads per core
MAX_POS = 4096
ROPE_THETA = 10000.0
SCALE = float(1.0 / np.sqrt(np.float32(D)))

_compiled = None
_last_in_maps = None


# ---------------------------------------------------------------- device ----
def _build_program():
    import concourse.bass as bass  # noqa: F401
    import concourse.mybir as mybir
    from concourse import bacc
    from concourse.tile import TileContext

    F32R = mybir.dt.float32r
    F32 = mybir.dt.float32
    BF16 = mybir.dt.bfloat16
    AF = mybir.ActivationFunctionType

    nc = bacc.Bacc("TRN2", target_bir_lowering=False, debug=False)
    xtq = nc.dram_tensor("xtq", [16, 128, 2, S], BF16, kind="ExternalInput").ap()
    xtk = nc.dram_tensor("xtk", [16, 128, 2, S], BF16, kind="ExternalInput").ap()
    xtv = nc.dram_tensor("xtv", [16, 128, 2, S], BF16, kind="ExternalInput").ap()
    wq = nc.dram_tensor("wq", [8, 128, 4, HQ * D], BF16, kind="ExternalInput").ap()
    wk = nc.dram_tensor("wk", [8, 128, 4, NKV * D], BF16, kind="ExternalInput").ap()
    wv = nc.dram_tensor("wv", [8, 128, 4, NKV * D], BF16, kind="ExternalInput").ap()
    wo = nc.dram_tensor("wo", [16, 128, 2, HQ * D], BF16, kind="ExternalInput").ap()
    cost = nc.dram_tensor("cost", [D, S], BF16, kind="ExternalInput").ap()
    ssin = nc.dram_tensor("ssin", [D, S], BF16, kind="ExternalInput").ap()
    masks = nc.dram_tensor("masks", [D, 512], BF16, kind="ExternalInput").ap()
    outT = nc.dram_tensor("outT", [E, S], BF16, kind="ExternalOutput").ap()

    with TileContext(nc) as tc:
        import contextlib
        with contextlib.ExitStack() as ctx:
            pers = ctx.enter_context(tc.tile_pool(name="pers", bufs=1))
            xs = ctx.enter_context(tc.tile_pool(name="xs", bufs=3))
            ws = ctx.enter_context(tc.tile_pool(name="ws", bufs=2))
            wop = ctx.enter_context(tc.tile_pool(name="wop", bufs=2))
            expp = ctx.enter_context(tc.tile_pool(name="expp", bufs=4))
            ropet = ctx.enter_context(tc.tile_pool(name="ropet", bufs=2))
            denp = ctx.enter_context(tc.tile_pool(name="denp", bufs=2))
            recp = ctx.enter_context(tc.tile_pool(name="recp", bufs=2))
            ostp = ctx.enter_context(tc.tile_pool(name="ostp", bufs=2))
            psp = ctx.enter_context(tc.tile_pool(name="psp", bufs=4, space="PSUM"))
            PBUFS = {"A": 4, "B": 2, "C": 2}

            # ---- persistent tiles
            kt_sb = pers.tile([128, NKV * S], BF16, tag="kt")      # K^T
            v_sb = pers.tile([128, 16 * NKV, D], BF16, tag="v")    # V s-tiles
            wv_sb = pers.tile([128, 32, NKV * D], BF16, tag="wv")
            xqh = pers.tile([128, 32, 1024], BF16, tag="xqh")      # x half
            qth = pers.tile([128, HQ * 1024], BF16, tag="qth")
            ctxh = pers.tile([128, HQ * 1024], BF16, tag="ctxh")
            tcos = pers.tile([128, S], BF16, tag="cos")
            tsin = pers.tile([128, S], BF16, tag="sin")
            tmsk = pers.tile([128, 512], BF16, tag="msk")
            ones_f = pers.tile([128, 128], F32, tag="onesf")
            ones = pers.tile([128, 128], F32R, tag="ones")

            def rope_evict(ps, coff, dst):
                """dst(bf16) = ps*cos + rot_half(ps)*sin, cols [coff,coff+512).

                ACT copies PSUM->SBUF bf16 first: the bank frees after
                ~0.43us and the DVE chain runs all-bf16 at 2x rate."""
                stage = ropet.tile([128, 512], BF16, tag="stage", bufs=4)
                nc.scalar.activation(stage[:], ps[:], AF.Copy)
                t2 = ropet.tile([128, 512], BF16, tag="t2")
                tc_ = ropet.tile([128, 512], BF16, tag="tc")
                nc.vector.tensor_mul(t2[0:64, :], stage[64:128, :],
                                     tsin[64:128, coff:coff + 512])
                nc.vector.tensor_mul(t2[64:128, :], stage[0:64, :],
                                     tsin[0:64, coff:coff + 512])
                nc.vector.tensor_mul(tc_[:], stage[:],
                                     tcos[:, coff:coff + 512])
                nc.vector.tensor_add(dst, tc_[:], t2[:])

            # ---- A-k merged groups (2 ets each), interleaved into aq0 ----
            ak_state = {}

            def ak_group(k):
                """k in 0..31: gen g = k//16 (s-half), et-pair i = k%16."""
                g, i = k // 16, k % 16
                s0 = g * 1024
                if i == 0:
                    ak_state["ps"] = [
                        psp.tile([128, 512], F32, tag="A", bufs=4,
                                 name=f"ps_k{g}_{j}") for j in range(4)]
                if i % 2 == 0:
                    ak_state["wk"] = ws.tile([128, 4, NKV * D], BF16, tag="wk", name="wk_t")
                    nc.scalar.dma_start(ak_state["wk"][:], wk[i // 2])
                ps_k, wk_t = ak_state["ps"], ak_state["wk"]
                xk_t = xs.tile([128, 2, 1024], BF16, tag="xk", bufs=2)
                nc.sync.dma_start(xk_t[:], xtk[i, :, :, s0:s0 + 1024])
                for e2 in range(2):
                    for db in range(NKV):
                        for sc in range(2):
                            nc.tensor.matmul(
                                ps_k[db * 2 + sc][:],
                                wk_t[:, (2 * i + e2) % 4,
                                     db * 128:(db + 1) * 128],
                                xk_t[:, e2, sc * 512:(sc + 1) * 512],
                                start=(i == 0 and e2 == 0),
                                stop=(i == 15 and e2 == 1))
                if i == 15:
                    for db in range(NKV):
                        for sc in range(2):
                            coff = s0 + sc * 512
                            rope_evict(ps_k[db * 2 + sc][:], coff,
                                       kt_sb[:, db * S + coff:
                                             db * S + coff + 512])

            # ---- misc loads spread into aq_phase(0) ----
            def misc_load(step):
                if step == 6:
                    nc.sync.dma_start(tcos[:, 0:1024], cost[:, 0:1024])
                    nc.gpsimd.memset(ones_f[:], 1.0)
                    nc.vector.tensor_copy(ones[:], ones_f[:])
                elif step == 10:
                    nc.sync.dma_start(tsin[:, 0:1024], ssin[:, 0:1024])
                elif step == 66:
                    nc.scalar.dma_start(tcos[:, 1024:2048], cost[:, 1024:2048])
                    nc.scalar.dma_start(tsin[:, 1024:2048], ssin[:, 1024:2048])
                    nc.scalar.dma_start(tmsk[:], masks[:])
                elif 75 <= step < 107 and (step - 75) % 4 == 0:
                    j = (step - 75) // 4
                    nc.gpsimd.dma_start(wv_sb[:, 4 * j:4 * j + 4, :], wv[j])

            # ---- A-q phase: 4 gens x 4 banks per half ----
            def aq_phase(half, interleave=None, ptags=("B", "B", "C", "C")):
                """gens: (h0-3,qp0),(h0-3,qp1),(h4-7,qp0),(h4-7,qp1)."""
                step = 0
                for gen in range(4):
                    hh0, qp = (gen // 2) * 4, gen % 2
                    ps_q = [psp.tile([128, 512], F32, tag=ptags[i],
                                     bufs=PBUFS[ptags[i]],
                                     name=f"ps_q{i}") for i in range(4)]
                    wq_t = None
                    for et in range(32):
                        if gen < 2 and et % 2 == 0:
                            q0 = half * 1024 + qp * 512
                            if gen == 0 and et == 0:
                                # split: first matmul starts ~2us earlier
                                nc.sync.dma_start(
                                    xqh[:, 0:1, 0:512],
                                    xtq[0, :, 0:1, q0:q0 + 512])
                                nc.sync.dma_start(
                                    xqh[:, 1:2, 0:512],
                                    xtq[0, :, 1:2, q0:q0 + 512])
                            else:
                                nc.sync.dma_start(
                                    xqh[:, et:et + 2, qp * 512:qp * 512 + 512],
                                    xtq[et // 2, :, :, q0:q0 + 512])
                        if et % 4 == 0:
                            wq_t = ws.tile([128, 4, 512], BF16, tag="wqp")
                            wslc = wq[et // 4, :, :, hh0 * 128: hh0 * 128 + 512]
                            if gen == 0 and et == 0:
                                nc.scalar.dma_start(wq_t[:, 0:1, :], wslc[:, 0:1, :])
                                nc.scalar.dma_start(wq_t[:, 1:4, :], wslc[:, 1:4, :])
                            else:
                                nc.scalar.dma_start(wq_t[:], wslc)
                        for hq in range(4):
                            nc.tensor.matmul(
                                ps_q[hq][:],
                                wq_t[:, et % 4, hq * 128:(hq + 1) * 128],
                                xqh[:, et, qp * 512:(qp + 1) * 512],
                                start=(et == 0), stop=(et == 31))
                        if interleave is not None:
                            interleave(step)
                        step += 1
                    for hq in range(4):
                        coff = half * 1024 + qp * 512
                        dcol = (hh0 + hq) * 1024 + qp * 512
                        rope_evict(ps_q[hq][:], coff,
                                   qth[:, dcol:dcol + 512])

            # ---- A-v stage (4 banks, 2-et merged loads), split emitter ----
            def av_make(stg):
                """Returns emit(i), i in 0..15, one 2-et group per call."""
                state = {}

                def emit(i):
                    if i == 0:
                        state["ps"] = [psp.tile([128, NKV * D], F32, tag="A",
                                                bufs=4,
                                                name=f"ps_v{stg}_{j}")
                                       for j in range(4)]
                    ps_v = state["ps"]
                    xv_t = xs.tile([128, 2, 512], BF16, tag="xv", bufs=4)
                    nc.sync.dma_start(
                        xv_t[:], xtv[i, :, :, stg * 512:(stg + 1) * 512])
                    for e2 in range(2):
                        for sti in range(4):
                            nc.tensor.matmul(
                                ps_v[sti][:],
                                xv_t[:, e2, sti * 128:(sti + 1) * 128],
                                wv_sb[:, 2 * i + e2, :],
                                start=(i == 0 and e2 == 0),
                                stop=(i == 15 and e2 == 1))
                    if i == 15:
                        for sti in range(4):
                            st = stg * 4 + sti
                            dst = v_sb[:, st * NKV:(st + 1) * NKV, :]
                            nc.vector.tensor_copy(dst, ps_v[sti][:])
                return emit

            def av_stage(stg):
                emit = av_make(stg)
                for i in range(16):
                    emit(i)

            # ---- attention block (one 512-query block) ----
            # Heads run in pairs (the 2nd head's matmuls cover the 1st
            # head's exp chain); interleave() supplies co-scheduled work
            # (A-v groups / C-phase groups) that also covers the
            # denominator flush at each pair boundary.
            def b_flush(pend, qtl):
                for h in pend["heads"]:
                    ps_dn = psp.tile([128, 512], F32, tag="B", bufs=2)
                    nc.tensor.matmul(ps_dn[:], ones[:], pend["dn"][h][:],
                                     start=True, stop=True)
                    rec = recp.tile([128, 512], F32, tag="rec")
                    nc.vector.reciprocal(rec[:], ps_dn[:])
                    dcol = h * 1024 + qtl * 512
                    nc.vector.tensor_mul(ctxh[:, dcol:dcol + 512],
                                         pend["cu"][h][:], rec[:])

            def b_block(qt, units=None):
                half, qtl = qt // 2, qt % 2
                nkt = 4 * (qt + 1)
                total = (HQ // 2) * nkt
                units = units or []
                done = 0
                step = 0
                for hp in range(HQ // 2):
                    heads = (2 * hp, 2 * hp + 1)
                    kv = heads[0] // GROUPS
                    ps_cu = {h: psp.tile([128, 512], F32, tag="C", bufs=2,
                                         name=f"ps_cu{h}") for h in heads}
                    acc_dn = {h: denp.tile([128, 512], F32R, tag="dn",
                                           name=f"acc_dn{h}", bufs=3)
                              for h in heads}
                    for kt in range(nkt):
                        j = kt - 4 * qt
                        c0 = max(j, 0) * 128   # masked-zero q-cols skipped
                        w = 512 - c0
                        tes = {}
                        for h in heads:
                            ps_s = psp.tile([128, 512], F32, tag="B", bufs=2)
                            nc.tensor.matmul(
                                ps_s[:, c0:512],
                                kt_sb[:, kv * S + kt * 128:
                                      kv * S + (kt + 1) * 128],
                                qth[:, h * 1024 + qtl * 512 + c0:
                                    h * 1024 + (qtl + 1) * 512],
                                start=True, stop=True)
                            te = expp.tile([128, 512], BF16, tag="expS")
                            nc.scalar.activation(te[:, c0:512],
                                                 ps_s[:, c0:512],
                                                 AF.Exp, scale=SCALE)
                            if j >= 0:
                                nc.vector.tensor_mul(
                                    te[:, c0:512], te[:, c0:512],
                                    tmsk[:, 0:w])
                            tes[h] = te
                        for h in heads:
                            te = tes[h]
                            nc.tensor.matmul(
                                ps_cu[h][:, c0:512],
                                v_sb[:, kt * NKV + kv, :],
                                te[:, c0:512],
                                start=(kt == 0), stop=(kt == nkt - 1))
                            if kt == 0:
                                nc.vector.tensor_copy(acc_dn[h][:], te[:])
                            else:
                                nc.vector.tensor_add(acc_dn[h][:, c0:512],
                                                     acc_dn[h][:, c0:512],
                                                     te[:, c0:512])
                        step += 1
                        target = step * len(units) // total
                        while done < target:
                            units[done]()
                            done += 1
                    b_flush({"heads": heads, "cu": ps_cu, "dn": acc_dn}, qtl)
                while done < len(units):
                    units[done]()
                    done += 1

            # ---- C phase merged groups (2 e32s each) ----
            def c_group(half, i16, sls=(0, 1), ptags=("B", "C", "B", "C")):
                wo_t = {}
                for e2 in range(2):
                    wo_t[e2] = wop.tile([128, HQ * 128], BF16, tag="wo1",
                                        bufs=4, name=f"wo_t{e2}")
                    nc.sync.dma_start(wo_t[e2][:], wo[i16][:, e2, :])
                nsl = len(sls)
                ps_o = {}
                k = 0
                for e2 in range(2):
                    for sl in sls:
                        ps_o[(e2, sl)] = psp.tile(
                            [128, 512], F32, tag=ptags[k],
                            bufs=PBUFS[ptags[k]], name=f"ps_o{k}")
                        k += 1
                for hd in range(HQ):
                    for e2 in range(2):
                        for sl in sls:
                            nc.tensor.matmul(
                                ps_o[(e2, sl)][:],
                                wo_t[e2][:, hd * 128:(hd + 1) * 128],
                                ctxh[:, hd * 1024 + sl * 512:
                                     hd * 1024 + (sl + 1) * 512],
                                start=(hd == 0), stop=(hd == HQ - 1))
                sg = half * 1024 + sls[0] * 512
                for e2 in range(2):
                    ost = ostp.tile([128, 1024], BF16, tag="ost")
                    for si, sl in enumerate(sls):
                        nc.scalar.activation(
                            ost[:, si * 512:(si + 1) * 512],
                            ps_o[(e2, sl)][:], AF.Copy)
                    e32 = 2 * i16 + e2
                    nc.gpsimd.dma_start(
                        outT[e32 * 128:(e32 + 1) * 128,
                             sg:sg + nsl * 512],
                        ost[:, 0:nsl * 512])

            # ---- schedule ----
            def interleave_ak(step):
                misc_load(step)
                if step >= 64 and step % 2 == 1:
                    ak_group((step - 64) // 2)

            def av_units(stg, lo=0, hi=16):
                emit = av_make(stg)
                return [(lambda i=i, e=emit: e(i)) for i in range(lo, hi)]

            def interleave_c0(step):
                if step % 8 == 2:
                    c_group(0, step // 8)

            aq_phase(0, interleave=interleave_ak)
            av_stage(0)
            b_block(0, units=av_units(1))
            b_block(1, units=av_units(2))
            aq_phase(1, interleave=interleave_c0, ptags=("A",) * 4)
            b_block(2, units=av_units(3))
            b_block(3, units=[(lambda i=i: c_group(1, i, sls=(0,),
                                                   ptags=("A", "A")))
                              for i in range(16)])
            for i16 in range(16):
                c_group(1, i16, sls=(1,), ptags=("A", "A"))

    nc.compile()
    return nc


def _get_program():
    global _compiled
    if _compiled is None:
        _compiled = _build_program()
    return _compiled


# ------------------------------------------------------------------ host ----
def _rope_tables_np():
    """Replicate reference._rope_tables in float32 numpy."""
    j = np.arange(0, D, 2, dtype=np.float32)
    inv_freq = (np.float32(1.0) / (np.float32(ROPE_THETA) ** (j / np.float32(D)))
                ).astype(np.float32)
    t = np.arange(MAX_POS, dtype=np.float32)
    freqs = (t[:, None] * inv_freq[None, :]).astype(np.float32)  # [max_pos, D/2]
    emb = np.concatenate([freqs, freqs], axis=-1)                # [max_pos, D]
    return np.cos(emb).astype(np.float32), np.sin(emb).astype(np.float32)


def _numpy_fallback(query, key, value, position_ids, src_mask, Wq, Wk, Wv, Wo):
    cos_t, sin_t = _rope_tables_np()
    pos = np.asarray(position_ids).astype(np.int64)
    cos = cos_t[pos][:, None]
    sin = sin_t[pos][:, None]
    nb, q_len, _ = query.shape
    q = (query @ Wq).reshape(nb, q_len, H, D).transpose(0, 2, 1, 3)
    k = (key @ Wk).reshape(nb, q_len, KVH, D).transpose(0, 2, 1, 3)
    v = (value @ Wv).reshape(nb, q_len, KVH, D).transpose(0, 2, 1, 3)

    def rot(x):
        return np.concatenate([-x[..., D // 2:], x[..., :D // 2]], axis=-1)
    q = q * cos + rot(q) * sin
    k = k * cos + rot(k) * sin
    k = np.repeat(k, GROUPS, axis=1)
    v = np.repeat(v, GROUPS, axis=1)
    out = np.zeros((nb, q_len, E), np.float32)
    for b in range(nb):
        for h in range(H):
            s = (q[b, h] @ k[b, h].T) / np.sqrt(np.float32(D))
            s = np.where(src_mask[b] == 0, np.float32(-1e9), s)
            s = s - s.max(-1, keepdims=True)
            e = np.exp(s)
            a = e / e.sum(-1, keepdims=True)
            ctx = a @ v[b, h]
            out[b] += ctx @ Wo[h * D:(h + 1) * D, :]
    return out


def kernel(query, key, value, position_ids, src_mask, Wq, Wk, Wv, Wo):
    import ml_dtypes
    BF = ml_dtypes.bfloat16
    query = np.asarray(query, dtype=np.float32)
    key = np.asarray(key, dtype=np.float32)
    value = np.asarray(value, dtype=np.float32)
    Wq = np.asarray(Wq, dtype=np.float32)
    Wk = np.asarray(Wk, dtype=np.float32)
    Wv = np.asarray(Wv, dtype=np.float32)
    Wo = np.asarray(Wo, dtype=np.float32)
    pos = np.asarray(position_ids).astype(np.int64)
    mask = np.asarray(src_mask)

    causal = np.array_equal(
        mask[0], np.tril(np.ones((S, S), mask.dtype)))
    if causal and mask.shape[0] > 1:
        causal = all(np.array_equal(mask[b], mask[0]) for b in range(1, mask.shape[0]))
    if not causal or query.shape != (B, S, E):
        return _numpy_fallback(query, key, value, pos, mask, Wq, Wk, Wv, Wo)

    from concourse.bass_utils import run_bass_kernel_spmd
    nc = _get_program()

    cos_t, sin_t = _rope_tables_np()
    # single diagonal mask triangle: mask[rk, cq] = 1 if cq >= rk
    rk = np.arange(128)[:, None]
    cq = np.arange(512)[None, :]
    mpat = (cq >= rk).astype(BF)

    def et2(xT):
        """[E, S] -> [16, 128, 2, S] (2-et merged transfer layout)."""
        return np.ascontiguousarray(
            xT.reshape(16, 2, 128, S).transpose(0, 2, 1, 3))

    def et4(w):
        """[E, C] -> [8, 128, 4, C]."""
        c = w.shape[1]
        return np.ascontiguousarray(
            w.reshape(8, 4, 128, c).transpose(0, 2, 1, 3))

    in_maps = []
    per_batch = {}
    for b in range(B):
        cosT = np.ascontiguousarray(cos_t[pos[b]].T)         # [D, S]
        sinT = np.ascontiguousarray(sin_t[pos[b]].T)         # [D, S]
        # ssin: rows 0:64 = +sin, rows 64:128 = -sin (see rope_evict)
        ssin = np.concatenate([sinT[:64], -sinT[64:]], axis=0).astype(np.float32)
        per_batch[b] = {
            "xtq": et2(query[b].T.astype(BF)),
            "xtk": et2(key[b].T.astype(BF)),
            "xtv": et2(value[b].T.astype(BF)),
            "cost": cosT.astype(BF),
            "ssin": ssin.astype(BF),
        }
    for c in range(N_CORES):
        b, g = c // TP, c % TP
        wo_b = (Wo[g * HQ * D:(g + 1) * HQ * D, :]
                .reshape(HQ, 128, 32, 128).transpose(2, 1, 0, 3)
                .reshape(32, 128, HQ * 128))            # [e32, d, hd*128+ep]
        in_maps.append({
            **per_batch[b],
            "wq": et4(Wq[:, g * HQ * D:(g + 1) * HQ * D].astype(BF)),
            "wk": et4(Wk[:, g * NKV * D:(g + 1) * NKV * D].astype(BF)),
            "wv": et4(Wv[:, g * NKV * D:(g + 1) * NKV * D].astype(BF)),
            "wo": np.ascontiguousarray(
                wo_b.reshape(16, 2, 128, HQ * 128)
                .transpose(0, 2, 1, 3)).astype(BF),
            "masks": mpat,
        })

    global _last_in_maps
    _last_in_maps = in_maps
    res = run_bass_kernel_spmd(nc, in_maps, core_ids=list(range(N_CORES)))
    out = np.empty((B, S, E), np.float32)
    for b in range(B):
        acc = res.results[b * TP]["outT"].astype(np.float32)
        for g in range(1, TP):
            acc += res.results[b * TP + g]["outT"].astype(np.float32)
        out[b] = acc.T
    return out


if __name__ == "__main__":
    print("building program...")
    _get_program()
    print("built")


# revision 34
# speedup vs baseline: 1.2515x; 1.2515x over previous
"""Mistral GQA attention on 8 Trainium2 NeuronCores (bf16 v2).

Sharding: core c -> batch b = c//4, head-group g = c%4.
Each core computes 8 query heads (g*8..g*8+8) and their 2 KV heads
(g*2, g*2+1) for its batch over the full sequence, plus the partial
output projection for its head rows of Wo. Host sums the 4 partial
outputs per batch.

Optimizations vs the fp32r baseline:
- all matmul operands bf16 (fp32 PSUM accumulation); host converts.
  Same 1 cycle/row PE rate as fp32r@>=256-wide, but halves DMA/SBUF
  and removes the <256-wide 4x penalty on diagonal tiles.
- merged multi-et DMA transfers via 4D host layouts (HWDGE charges a
  fixed ~625ns per transfer on one shared device), split across the
  SP / ACT(HWDGE) and Pool(SWDGE) queues to avoid head-of-line blocks.
- every phase is co-scheduled with a complementary phase so the PE
  never idles (idle resets the PE p-state: 2x slower rows for 3us):
  A-k inside A-q half 0; C half 0 inside A-q half 1; A-v stages inside
  the (ACT-exp-bound) attention blocks b0-b2; C half 1 q-block 2
  inside b3. PSUM is split into tags A(4)/B(2)/C(2) so co-scheduled
  phases own disjoint banks and drains overlap accumulation.
- softmax denominator via DVE accumulation of exp tiles + one
  ones-matmul per (head, q-block) instead of a ones-matmul per k-tile;
  attention heads run in pairs to cover the exp chain latency.
- wv/xq-half resident in SBUF (no x re-reads); bf16 output partials
  summed on host.
"""
import numpy as np

B, S, E = 2, 2048, 4096
H, KVH, D = 32, 8, 128
GROUPS = H // KVH
N_CORES, TP = 8, 4
HQ = H // TP          # 8 q heads per core
NKV = KVH // TP       # 2 kv heads per core
MAX_POS = 4096
ROPE_THETA = 10000.0
SCALE = float(1.0 / np.sqrt(np.float32(D)))

_compiled = None
_last_in_maps = None


# ---------------------------------------------------------------- device ----
def _build_program():
    import concourse.bass as bass  # noqa: F401
    import concourse.mybir as mybir
    from concourse import bacc
    from concourse.tile import TileContext

    F32R = mybir.dt.float32r
    F32 = mybir.dt.float32
    BF16 = mybir.dt.bfloat16
    AF = mybir.ActivationFunctionType

    nc = bacc.Bacc("TRN2", target_bir_lowering=False, debug=False)
    xtq = nc.dram_tensor("xtq", [16, 128, 2, S], BF16, kind="ExternalInput").ap()
    xtk = nc.dram_tensor("xtk", [16, 128, 2, S], BF16, kind="ExternalInput").ap()
    xtv = nc.dram_tensor("xtv", [16, 128, 2, S], BF16, kind="ExternalInput").ap()
    wq = nc.dram_tensor("wq", [8, 128, 4, HQ * D], BF16, kind="ExternalInput").ap()
    wk = nc.dram_tensor("wk", [8, 128, 4, NKV * D], BF16, kind="ExternalInput").ap()
    wv = nc.dram_tensor("wv", [8, 128, 4, NKV * D], BF16, kind="ExternalInput").ap()
    wo = nc.dram_tensor("wo", [16, 128, 2, HQ * D], BF16, kind="ExternalInput").ap()
    cost = nc.dram_tensor("cost", [D, S], BF16, kind="ExternalInput").ap()
    ssin = nc.dram_tensor("ssin", [D, S], BF16, kind="ExternalInput").ap()
    masks = nc.dram_tensor("masks", [D, 512], BF16, kind="ExternalInput").ap()
    outT = nc.dram_tensor("outT", [E, S], BF16, kind="ExternalOutput").ap()

    with TileContext(nc) as tc:
        import contextlib
        with contextlib.ExitStack() as ctx:
            pers = ctx.enter_context(tc.tile_pool(name="pers", bufs=1))
            xs = ctx.enter_context(tc.tile_pool(name="xs", bufs=3))
            ws = ctx.enter_context(tc.tile_pool(name="ws", bufs=2))
            wop = ctx.enter_context(tc.tile_pool(name="wop", bufs=2))
            expp = ctx.enter_context(tc.tile_pool(name="expp", bufs=6))
            ropet = ctx.enter_context(tc.tile_pool(name="ropet", bufs=2))
            denp = ctx.enter_context(tc.tile_pool(name="denp", bufs=2))
            recp = ctx.enter_context(tc.tile_pool(name="recp", bufs=2))
            ostp = ctx.enter_context(tc.tile_pool(name="ostp", bufs=2))
            psp = ctx.enter_context(tc.tile_pool(name="psp", bufs=4, space="PSUM"))
            PBUFS = {"A": 4, "B": 2, "C": 2}

            # ---- persistent tiles
            kt_sb = pers.tile([128, NKV * S], BF16, tag="kt")      # K^T
            v_sb = pers.tile([128, 16 * NKV, D], BF16, tag="v")    # V s-tiles
            wv_sb = pers.tile([128, 32, NKV * D], BF16, tag="wv")
            xqh = pers.tile([128, 32, 1024], BF16, tag="xqh")      # x half
            qth = pers.tile([128, HQ * 1024], BF16, tag="qth")
            ctxh = pers.tile([128, HQ * 1024], BF16, tag="ctxh")
            tcos = pers.tile([128, S], BF16, tag="cos")
            tsin = pers.tile([128, S], BF16, tag="sin")
            tmsk = pers.tile([128, 512], BF16, tag="msk")
            ones_f = pers.tile([128, 128], F32, tag="onesf")
            ones = pers.tile([128, 128], F32R, tag="ones")

            def rope_evict(ps, coff, dst):
                """dst(bf16) = ps*cos + rot_half(ps)*sin, cols [coff,coff+512).

                ACT copies PSUM->SBUF bf16 first: the bank frees after
                ~0.43us and the DVE chain runs all-bf16 at 2x rate."""
                stage = ropet.tile([128, 512], BF16, tag="stage", bufs=4)
                nc.scalar.activation(stage[:], ps[:], AF.Copy)
                t2 = ropet.tile([128, 512], BF16, tag="t2")
                tc_ = ropet.tile([128, 512], BF16, tag="tc")
                nc.vector.tensor_mul(t2[0:64, :], stage[64:128, :],
                                     tsin[64:128, coff:coff + 512])
                nc.vector.tensor_mul(t2[64:128, :], stage[0:64, :],
                                     tsin[0:64, coff:coff + 512])
                nc.vector.tensor_mul(tc_[:], stage[:],
                                     tcos[:, coff:coff + 512])
                nc.vector.tensor_add(dst, tc_[:], t2[:])

            # ---- A-k merged groups (2 ets each), interleaved into aq0 ----
            ak_state = {}

            def ak_group(k):
                """k in 0..31: gen g = k//16 (s-half), et-pair i = k%16."""
                g, i = k // 16, k % 16
                s0 = g * 1024
                if i == 0:
                    ak_state["ps"] = [
                        psp.tile([128, 512], F32, tag="A", bufs=4,
                                 name=f"ps_k{g}_{j}") for j in range(4)]
                if i % 2 == 0:
                    ak_state["wk"] = ws.tile([128, 4, NKV * D], BF16, tag="wk", name="wk_t")
                    nc.scalar.dma_start(ak_state["wk"][:], wk[i // 2])
                ps_k, wk_t = ak_state["ps"], ak_state["wk"]
                xk_t = xs.tile([128, 2, 1024], BF16, tag="xk", bufs=2)
                nc.sync.dma_start(xk_t[:], xtk[i, :, :, s0:s0 + 1024])
                for e2 in range(2):
                    for db in range(NKV):
                        for sc in range(2):
                            nc.tensor.matmul(
                                ps_k[db * 2 + sc][:],
                                wk_t[:, (2 * i + e2) % 4,
                                     db * 128:(db + 1) * 128],
                                xk_t[:, e2, sc * 512:(sc + 1) * 512],
                                start=(i == 0 and e2 == 0),
                                stop=(i == 15 and e2 == 1))
                if i == 15:
                    for db in range(NKV):
                        for sc in range(2):
                            coff = s0 + sc * 512
                            rope_evict(ps_k[db * 2 + sc][:], coff,
                                       kt_sb[:, db * S + coff:
                                             db * S + coff + 512])

            # ---- misc loads spread into aq_phase(0) ----
            def misc_load(step):
                if step == 6:
                    nc.sync.dma_start(tcos[:, 0:1024], cost[:, 0:1024])
                    nc.gpsimd.memset(ones_f[:], 1.0)
                    nc.vector.tensor_copy(ones[:], ones_f[:])
                elif step == 10:
                    nc.sync.dma_start(tsin[:, 0:1024], ssin[:, 0:1024])
                elif step == 66:
                    nc.scalar.dma_start(tcos[:, 1024:2048], cost[:, 1024:2048])
                    nc.scalar.dma_start(tsin[:, 1024:2048], ssin[:, 1024:2048])
                    nc.scalar.dma_start(tmsk[:], masks[:])
                elif 75 <= step < 107 and (step - 75) % 4 == 0:
                    j = (step - 75) // 4
                    nc.gpsimd.dma_start(wv_sb[:, 4 * j:4 * j + 4, :], wv[j])

            # ---- A-q phase: 4 gens x 4 banks per half ----
            def aq_phase(half, interleave=None, ptags=("B", "B", "C", "C")):
                """gens: (h0-3,qp0),(h0-3,qp1),(h4-7,qp0),(h4-7,qp1)."""
                step = 0
                for gen in range(4):
                    hh0, qp = (gen // 2) * 4, gen % 2
                    ps_q = [psp.tile([128, 512], F32, tag=ptags[i],
                                     bufs=PBUFS[ptags[i]],
                                     name=f"ps_q{i}") for i in range(4)]
                    wq_t = None
                    for et in range(32):
                        if gen < 2 and et % 2 == 0:
                            q0 = half * 1024 + qp * 512
                            if gen == 0 and et == 0:
                                # split: first matmul starts ~2us earlier
                                nc.sync.dma_start(
                                    xqh[:, 0:1, 0:512],
                                    xtq[0, :, 0:1, q0:q0 + 512])
                                nc.sync.dma_start(
                                    xqh[:, 1:2, 0:512],
                                    xtq[0, :, 1:2, q0:q0 + 512])
                            else:
                                nc.sync.dma_start(
                                    xqh[:, et:et + 2, qp * 512:qp * 512 + 512],
                                    xtq[et // 2, :, :, q0:q0 + 512])
                        if et % 4 == 0:
                            wq_t = ws.tile([128, 4, 512], BF16, tag="wqp")
                            wslc = wq[et // 4, :, :, hh0 * 128: hh0 * 128 + 512]
                            if gen == 0 and et == 0:
                                nc.scalar.dma_start(wq_t[:, 0:1, :], wslc[:, 0:1, :])
                                nc.scalar.dma_start(wq_t[:, 1:4, :], wslc[:, 1:4, :])
                            else:
                                nc.scalar.dma_start(wq_t[:], wslc)
                        for hq in range(4):
                            nc.tensor.matmul(
                                ps_q[hq][:],
                                wq_t[:, et % 4, hq * 128:(hq + 1) * 128],
                                xqh[:, et, qp * 512:(qp + 1) * 512],
                                start=(et == 0), stop=(et == 31))
                        if interleave is not None:
                            interleave(step)
                        step += 1
                    for hq in range(4):
                        coff = half * 1024 + qp * 512
                        dcol = (hh0 + hq) * 1024 + qp * 512
                        rope_evict(ps_q[hq][:], coff,
                                   qth[:, dcol:dcol + 512])

            # ---- A-v stage (4 banks, 2-et merged loads), split emitter ----
            def av_make(stg):
                """Returns emit(i), i in 0..15, one 2-et group per call."""
                state = {}

                def emit(i):
                    if i == 0:
                        state["ps"] = [psp.tile([128, NKV * D], F32, tag="A",
                                                bufs=4,
                                                name=f"ps_v{stg}_{j}")
                                       for j in range(4)]
                    ps_v = state["ps"]
                    xv_t = xs.tile([128, 2, 512], BF16, tag="xv", bufs=4)
                    nc.sync.dma_start(
                        xv_t[:], xtv[i, :, :, stg * 512:(stg + 1) * 512])
                    for e2 in range(2):
                        for sti in range(4):
                            nc.tensor.matmul(
                                ps_v[sti][:],
                                xv_t[:, e2, sti * 128:(sti + 1) * 128],
                                wv_sb[:, 2 * i + e2, :],
                                start=(i == 0 and e2 == 0),
                                stop=(i == 15 and e2 == 1))
                    if i == 15:
                        for sti in range(4):
                            st = stg * 4 + sti
                            dst = v_sb[:, st * NKV:(st + 1) * NKV, :]
                            nc.vector.tensor_copy(dst, ps_v[sti][:])
                return emit

            def av_stage(stg):
                emit = av_make(stg)
                for i in range(16):
                    emit(i)

            # ---- attention block (one 512-query block) ----
            # Heads run in pairs (the 2nd head's matmuls cover the 1st
            # head's exp chain); interleave() supplies co-scheduled work
            # (A-v groups / C-phase groups) that also covers the
            # denominator flush at each pair boundary.
            def b_flush(pend, qtl):
                for h in pend["heads"]:
                    ps_dn = psp.tile([128, 512], F32, tag="B", bufs=2)
                    nc.tensor.matmul(ps_dn[:], ones[:], pend["dn"][h][:],
                                     start=True, stop=True)
                    rec = recp.tile([128, 512], F32, tag="rec")
                    nc.vector.reciprocal(rec[:], ps_dn[:])
                    dcol = h * 1024 + qtl * 512
                    nc.vector.tensor_mul(ctxh[:, dcol:dcol + 512],
                                         pend["cu"][h][:], rec[:])

            def b_block(qt, units=None):
                half, qtl = qt // 2, qt % 2
                nkt = 4 * (qt + 1)
                total = (HQ // 2) * nkt
                units = units or []
                done = 0
                step = 0
                for hp in range(HQ // 2):
                    heads = (2 * hp, 2 * hp + 1)
                    kv = heads[0] // GROUPS
                    ps_cu = {h: psp.tile([128, 512], F32, tag="C", bufs=2,
                                         name=f"ps_cu{h}") for h in heads}
                    acc_dn = {h: denp.tile([128, 512], F32R, tag="dn",
                                           name=f"acc_dn{h}", bufs=3)
                              for h in heads}
                    for kt in range(nkt):
                        j = kt - 4 * qt
                        c0 = max(j, 0) * 128   # masked-zero q-cols skipped
                        w = 512 - c0
                        tes = {}
                        for h in heads:
                            ps_s = psp.tile([128, 512], F32, tag="B", bufs=2)
                            nc.tensor.matmul(
                                ps_s[:, c0:512],
                                kt_sb[:, kv * S + kt * 128:
                                      kv * S + (kt + 1) * 128],
                                qth[:, h * 1024 + qtl * 512 + c0:
                                    h * 1024 + (qtl + 1) * 512],
                                start=True, stop=True)
                            te = expp.tile([128, 512], BF16, tag="expS")
                            nc.scalar.activation(te[:, c0:512],
                                                 ps_s[:, c0:512],
                                                 AF.Exp, scale=SCALE)
                            if j >= 0:
                                nc.vector.tensor_mul(
                                    te[:, c0:512], te[:, c0:512],
                                    tmsk[:, 0:w])
                            tes[h] = te
                        for h in heads:
                            te = tes[h]
                            nc.tensor.matmul(
                                ps_cu[h][:, c0:512],
                                v_sb[:, kt * NKV + kv, :],
                                te[:, c0:512],
                                start=(kt == 0), stop=(kt == nkt - 1))
                            if kt == 0:
                                nc.vector.tensor_copy(acc_dn[h][:], te[:])
                            else:
                                nc.vector.tensor_add(acc_dn[h][:, c0:512],
                                                     acc_dn[h][:, c0:512],
                                                     te[:, c0:512])
                        step += 1
                        target = step * len(units) // total
                        while done < target:
                            units[done]()
                            done += 1
                    b_flush({"heads": heads, "cu": ps_cu, "dn": acc_dn}, qtl)
                while done < len(units):
                    units[done]()
                    done += 1

            # ---- C phase merged groups (2 e32s each) ----
            def c_group(half, i16, sls=(0, 1), ptags=("B", "C", "B", "C")):
                wo_t = {}
                for e2 in range(2):
                    wo_t[e2] = wop.tile([128, HQ * 128], BF16, tag="wo1",
                                        bufs=4, name=f"wo_t{e2}")
                    nc.sync.dma_start(wo_t[e2][:], wo[i16][:, e2, :])
                nsl = len(sls)
                ps_o = {}
                k = 0
                for e2 in range(2):
                    for sl in sls:
                        ps_o[(e2, sl)] = psp.tile(
                            [128, 512], F32, tag=ptags[k],
                            bufs=PBUFS[ptags[k]], name=f"ps_o{k}")
                        k += 1
                for hd in range(HQ):
                    for e2 in range(2):
                        for sl in sls:
                            nc.tensor.matmul(
                                ps_o[(e2, sl)][:],
                                wo_t[e2][:, hd * 128:(hd + 1) * 128],
                                ctxh[:, hd * 1024 + sl * 512:
                                     hd * 1024 + (sl + 1) * 512],
                                start=(hd == 0), stop=(hd == HQ - 1))
                sg = half * 1024 + sls[0] * 512
                for e2 in range(2):
                    ost = ostp.tile([128, 1024], BF16, tag="ost")
                    for si, sl in enumerate(sls):
                        nc.scalar.activation(
                            ost[:, si * 512:(si + 1) * 512],
                            ps_o[(e2, sl)][:], AF.Copy)
                    e32 = 2 * i16 + e2
                    nc.gpsimd.dma_start(
                        outT[e32 * 128:(e32 + 1) * 128,
                             sg:sg + nsl * 512],
                        ost[:, 0:nsl * 512])

            # ---- schedule ----
            def interleave_ak(step):
                misc_load(step)
                if step >= 64 and step % 2 == 1:
                    ak_group((step - 64) // 2)

            def av_units(stg, lo=0, hi=16):
                emit = av_make(stg)
                return [(lambda i=i, e=emit: e(i)) for i in range(lo, hi)]

            def interleave_c0(step):
                if step % 8 == 2:
                    c_group(0, step // 8)

            aq_phase(0, interleave=interleave_ak)
            av_stage(0)
            b_block(0, units=av_units(1))
            b_block(1, units=av_units(2))
            aq_phase(1, interleave=interleave_c0, ptags=("A",) * 4)
            b_block(2, units=av_units(3))
            b_block(3, units=[(lambda i=i: c_group(1, i, sls=(0,),
                                                   ptags=("A", "A")))
                              for i in range(16)])
            for i16 in range(16):
                c_group(1, i16, sls=(1,), ptags=("A", "A"))

    nc.compile()
    return nc


def _get_program():
    global _compiled
    if _compiled is None:
        _compiled = _build_program()
    return _compiled


# ------------------------------------------------------------------ host ----
def _rope_tables_np():
    """Replicate reference._rope_tables in float32 numpy."""
    j = np.arange(0, D, 2, dtype=np.float32)
    inv_freq = (np.float32(1.0) / (np.float32(ROPE_THETA) ** (j / np.float32(D)))
                ).astype(np.float32)
    t = np.arange(MAX_POS, dtype=np.float32)
    freqs = (t[:, None] * inv_freq[None, :]).astype(np.float32)  # [max_pos, D/2]
    emb = np.concatenate([freqs, freqs], axis=-1)                # [max_pos, D]
    return np.cos(emb).astype(np.float32), np.sin(emb).astype(np.float32)


def _numpy_fallback(query, key, value, position_ids, src_mask, Wq, Wk, Wv, Wo):
    cos_t, sin_t = _rope_tables_np()
    pos = np.asarray(position_ids).astype(np.int64)
    cos = cos_t[pos][:, None]
    sin = sin_t[pos][:, None]
    nb, q_len, _ = query.shape
    q = (query @ Wq).reshape(nb, q_len, H, D).transpose(0, 2, 1, 3)
    k = (key @ Wk).reshape(nb, q_len, KVH, D).transpose(0, 2, 1, 3)
    v = (value @ Wv).reshape(nb, q_len, KVH, D).transpose(0, 2, 1, 3)

    def rot(x):
        return np.concatenate([-x[..., D // 2:], x[..., :D // 2]], axis=-1)
    q = q * cos + rot(q) * sin
    k = k * cos + rot(k) * sin
    k = np.repeat(k, GROUPS, axis=1)
    v = np.repeat(v, GROUPS, axis=1)
    out = np.zeros((nb, q_len, E), np.float32)
    for b in range(nb):
        for h in range(H):
            s = (q[b, h] @ k[b, h].T) / np.sqrt(np.float32(D))
            s = np.where(src_mask[b] == 0, np.float32(-1e9), s)
            s = s - s.max(-1, keepdims=True)
            e = np.exp(s)
            a = e / e.sum(-1, keepdims=True)
            ctx = a @ v[b, h]
            out[b] += ctx @ Wo[h * D:(h + 1) * D, :]
    return out


def kernel(query, key, value, position_ids, src_mask, Wq, Wk, Wv, Wo):
    import ml_dtypes
    BF = ml_dtypes.bfloat16
    query = np.asarray(query, dtype=np.float32)
    key = np.asarray(key, dtype=np.float32)
    value = np.asarray(value, dtype=np.float32)
    Wq = np.asarray(Wq, dtype=np.float32)
    Wk = np.asarray(Wk, dtype=np.float32)
    Wv = np.asarray(Wv, dtype=np.float32)
    Wo = np.asarray(Wo, dtype=np.float32)
    pos = np.asarray(position_ids).astype(np.int64)
    mask = np.asarray(src_mask)

    causal = np.array_equal(
        mask[0], np.tril(np.ones((S, S), mask.dtype)))
    if causal and mask.shape[0] > 1:
        causal = all(np.array_equal(mask[b], mask[0]) for b in range(1, mask.shape[0]))
    if not causal or query.shape != (B, S, E):
        return _numpy_fallback(query, key, value, pos, mask, Wq, Wk, Wv, Wo)

    from concourse.bass_utils import run_bass_kernel_spmd
    nc = _get_program()

    cos_t, sin_t = _rope_tables_np()
    # single diagonal mask triangle: mask[rk, cq] = 1 if cq >= rk
    rk = np.arange(128)[:, None]
    cq = np.arange(512)[None, :]
    mpat = (cq >= rk).astype(BF)

    def et2(xT):
        """[E, S] -> [16, 128, 2, S] (2-et merged transfer layout)."""
        return np.ascontiguousarray(
            xT.reshape(16, 2, 128, S).transpose(0, 2, 1, 3))

    def et4(w):
        """[E, C] -> [8, 128, 4, C]."""
        c = w.shape[1]
        return np.ascontiguousarray(
            w.reshape(8, 4, 128, c).transpose(0, 2, 1, 3))

    in_maps = []
    per_batch = {}
    for b in range(B):
        cosT = np.ascontiguousarray(cos_t[pos[b]].T)         # [D, S]
        sinT = np.ascontiguousarray(sin_t[pos[b]].T)         # [D, S]
        # ssin: rows 0:64 = +sin, rows 64:128 = -sin (see rope_evict)
        ssin = np.concatenate([sinT[:64], -sinT[64:]], axis=0).astype(np.float32)
        per_batch[b] = {
            "xtq": et2(query[b].T.astype(BF)),
            "xtk": et2(key[b].T.astype(BF)),
            "xtv": et2(value[b].T.astype(BF)),
            "cost": cosT.astype(BF),
            "ssin": ssin.astype(BF),
        }
    for c in range(N_CORES):
        b, g = c // TP, c % TP
        wo_b = (Wo[g * HQ * D:(g + 1) * HQ * D, :]
                .reshape(HQ, 128, 32, 128).transpose(2, 1, 0, 3)
                .reshape(32, 128, HQ * 128))            # [e32, d, hd*128+ep]
        in_maps.append({
            **per_batch[b],
            "wq": et4(Wq[:, g * HQ * D:(g + 1) * HQ * D].astype(BF)),
            "wk": et4(Wk[:, g * NKV * D:(g + 1) * NKV * D].astype(BF)),
            "wv": et4(Wv[:, g * NKV * D:(g + 1) * NKV * D].astype(BF)),
            "wo": np.ascontiguousarray(
                wo_b.reshape(16, 2, 128, HQ * 128)
                .transpose(0, 2, 1, 3)).astype(BF),
            "masks": mpat,
        })

    global _last_in_maps
    _last_in_maps = in_maps
    res = run_bass_kernel_spmd(nc, in_maps, core_ids=list(range(N_CORES)))
    out = np.empty((B, S, E), np.float32)
    for b in range(B):
        acc = res.results[b * TP]["outT"].astype(np.float32)
        for g in range(1, TP):
            acc += res.results[b * TP + g]["outT"].astype(np.float32)
        out[b] = acc.T
    return out


if __name__ == "__main__":
    print("building program...")
    _get_program()
    print("built")
